# revision 25
# baseline (speedup 1.0000x reference)
"""BlackwellLinear Trainium2 kernel: 2:4 sparsity + int8 fake-quant + x @ w.T + bias.

Full inputs in, full output out. Hybrid sharding across 8 NeuronCores:
4 token groups x 2 out_feature groups. Each core computes
y[tg-block, fg-block] = x[tg] @ w[fg].T * scale + bias[fg], and runs the
module's weight prep (2:4 sparsify + int8 fake-quant) for its own out_feature
half -- halving the elementwise prep work per core vs pure data-parallel
(that prep chain gates the single-pass matmul pipeline start). No collectives:
the global absmax over the sparsified weight equals absmax of |w| (the global
max always survives top-2-of-4 selection), and each core computes it from its
own fp32 half plus a compact fp16 shadow of the other half (scale perturbation
~2^-11, far inside the error budget).

Host does layout/encoding only: transposes, fp16 encodes of x / shadow / bias,
and a phase-major permutation of the in_features axis
(p <-> 4*(p%256) + p//256) applied to both x.T and w.T. The permutation makes
each group-of-4 (the 2:4 unit) span four k-tiles at the SAME partition/column
coordinates, so sparsify+quantize is contiguous full-width elementwise ops and
the quantized weight lands directly in [in_f, out_f] (lhsT) layout. All module
math (threshold, mask, quantize, matmul, bias) runs on device.

Engine layout of the prep, tuned from traces:
 - All weight DMAs issue before anything else; x-strip loads are fenced behind
   the last weight tile by a tiny dependent copy so they cannot crowd the wire
   during the prep-critical weight load. Bias rides inside the shadow tensor
   (fp16) so no separate DMA can get hoisted ahead of weights.
 - ACT: |w| for range-0 tiles, |shadow|, magic-round pairs for k-tiles != 0.
 - Pool: w*w for range-1 tiles (squares are order-isomorphic to |w| and
   fp32-tie-safe), mask-apply multiplies, the partition allreduce.
 - DVE: threshold trees (range 1 in the squared domain), masks (exact fp32
   compares -> reference-identical 2:4 tie behavior), absmax reduces (shadow
   via fp16 2x max-tree), PSUM evictions, and the WHOLE quantize chain for
   k-tile 0 (no cross-engine hops on the PE-gating tile).
 - PE: one fp16 pass -- 512 MMs of N=512, the dense roofline.

Numerics (harness gate rel_err < 2e-2; this kernel lands ~1e-3):
  s   = absmax * (1/qmax)                  (1 ulp from fl(absmax/qmax))
  inv ~= 1/s                               (HW reciprocal + 1 Newton step)
  k   = rne(w * inv)                       (magic-constant RNE round)
  q   = k * mask                           (2:4 mask; integers are fp16-exact)
  y   = s * (x16 @ q.T) + bias             (scale folded into PSUM eviction)
"""

import numpy as np

N_CORES = 8
P = 128
IN_F = 1024
OUT_F = 1024
TOKENS = 32768
T_GROUPS = 4
F_GROUPS = 2
TOK_PC = TOKENS // T_GROUPS  # 8192 tokens per core
OUT_PC = OUT_F // F_GROUPS  # 512 out_features per core
K_TILES = IN_F // P  # 8
M_TILES = OUT_PC // P  # 4
TB_TOK = 1024  # token block per x strip
N_TB = TOK_PC // TB_TOK  # 8
MM_N = 512  # matmul moving free dim (one PSUM bank of fp32)
TJ = TB_TOK // MM_N  # 2
SHW = K_TILES * OUT_PC  # 4096 shadow columns
WXW = SHW + M_TILES  # + fp16-packed bias columns

MAGIC = 12582912.0  # 1.5 * 2**23: (v + MAGIC) - MAGIC == RNE round, |v| <= 2**22

# k-tile order: range-0 tiles (phases of groups 0..127) first so the range-0
# threshold -> mask -> quant chain completes with half the weight DMA landed;
# PE accumulates k-tiles in this same order (sum order is commutative).
KT_ORDER = (0, 2, 4, 6, 1, 3, 5, 7)

# phase-major permutation of the in_features axis: position p holds original
# feature 4*(p%256) + p//256, so k-tile kt covers phase kt//2 of group range
# (kt%2)*128..+128 and the four phases of a group share partition/column coords
_PERM = (4 * (np.arange(IN_F) % 256) + np.arange(IN_F) // 256).astype(np.int64)

_CACHE = {}


def _build(qmax: float):
    from contextlib import ExitStack

    import concourse.tile as tile
    import concourse.mybir as mybir
    from concourse import bacc, bass_isa

    f32 = mybir.dt.float32
    f16 = mybir.dt.float16
    Alu = mybir.AluOpType
    Act = mybir.ActivationFunctionType

    inv_qmax = float(np.float32(1.0) / np.float32(qmax))

    nc = bacc.Bacc("TRN2", target_bir_lowering=False, debug=False)
    xth = nc.dram_tensor("xth", [IN_F, TOK_PC], f16, kind="ExternalInput").ap()
    # own out_f half of w.T (permuted in_f rows), fp32: exact 2:4 tie behavior
    wpo = nc.dram_tensor("wpo", [IN_F, OUT_PC], f32, kind="ExternalInput").ap()
    # other half, fp16 shadow packed [128, 4096] + fp16 bias [128, M_TILES]
    wpx = nc.dram_tensor("wpx", [P, WXW], f16, kind="ExternalInput").ap()
    yt = nc.dram_tensor("yt", [OUT_PC, TOK_PC], f16, kind="ExternalOutput").ap()

    with tile.TileContext(nc) as tc, ExitStack() as ctx:
        const = ctx.enter_context(tc.tile_pool(name="const", bufs=1))
        wnat_p = ctx.enter_context(tc.tile_pool(name="wnat", bufs=8))
        abs_p = ctx.enter_context(tc.tile_pool(name="absp", bufs=8))
        thr_p = ctx.enter_context(tc.tile_pool(name="thr", bufs=2))
        tt_p = ctx.enter_context(tc.tile_pool(name="ttmp", bufs=2))
        mask_p = ctx.enter_context(tc.tile_pool(name="mask", bufs=8))
        qtmp_p = ctx.enter_context(tc.tile_pool(name="qtmp", bufs=2))
        wqt_p = ctx.enter_context(tc.tile_pool(name="wqt", bufs=8))
        sc_p = ctx.enter_context(tc.tile_pool(name="sc", bufs=1))
        x_p = ctx.enter_context(tc.tile_pool(name="x", bufs=16))
        y_p = ctx.enter_context(tc.tile_pool(name="y", bufs=4))
        psum_mm = ctx.enter_context(tc.tile_pool(name="psmm", bufs=8, space="PSUM"))

        # ---- ALL weight DMAs first, shadow halves split across queues ----
        shx = const.tile([P, WXW], f16, tag="shx")
        SH2 = SHW // 2
        wk = [None] * K_TILES

        def wdma(eng, kt):
            wt = wnat_p.tile([P, OUT_PC], f32, tag="wnat", name=f"wnat{kt}")
            eng.dma_start(wt[:], wpo[kt * P : (kt + 1) * P, :])
            wk[kt] = wt

        wdma(nc.sync, 0)
        wdma(nc.scalar, 1)
        wdma(nc.sync, 2)
        wdma(nc.scalar, 3)
        wdma(nc.sync, 4)
        wdma(nc.scalar, 5)
        wdma(nc.sync, 6)
        wdma(nc.scalar, 7)
        nc.sync.dma_start(shx[:, 0:SH2], wpx[:, 0:SH2])
        nc.sync.dma_start(shx[:, SH2:WXW], wpx[:, SH2:WXW])

        # fence: x loads (emitted later on sync) stay behind the last weight
        # tile on the wire -- this copy cannot issue until wnat7 has landed
        gate = const.tile([1, 8], f32, tag="gate")
        nc.sync.dma_start(gate[:, :], wk[7][0:1, 0:8])

        # ---- ACT: |w| per k-tile (arrival-pipelined), then |shadow| ----
        ak = [None] * K_TILES
        for kt in KT_ORDER:
            a = abs_p.tile([P, OUT_PC], f32, tag="abs", name=f"abs{kt}")
            nc.scalar.activation(a[:], wk[kt][:], Act.Abs)
            ak[kt] = a
        ash = const.tile([P, SHW], f16, tag="ash")
        nc.scalar.activation(ash[:, 0:SH2], shx[:, 0:SH2], Act.Abs)
        nc.scalar.activation(ash[:, SH2:SHW], shx[:, SH2:SHW], Act.Abs)

        def vts(out, in0, s1, op0, s2=None, op1=None):
            kw = {"op1": op1} if op1 is not None else {}
            nc.vector.tensor_scalar(
                out=out, in0=in0, scalar1=s1, scalar2=s2, op0=op0, **kw
            )

        def vtt(out, in0, in1, op):
            nc.vector.tensor_tensor(out=out, in0=in0, in1=in1, op=op)

        # ---- DVE chain (ordered for earliest inv -> q16[kt0]) ----
        tA, tB = {}, {}

        def build_pair_max(r):
            a0, a1, a2, a3 = (ak[2 * j + r] for j in range(4))
            tA[r] = tt_p.tile([P, OUT_PC], f32, tag="tA", name=f"tA{r}")
            tB[r] = tt_p.tile([P, OUT_PC], f32, tag="tB", name=f"tB{r}")
            vtt(tA[r][:], a0[:], a1[:], Alu.max)
            vtt(tB[r][:], a2[:], a3[:], Alu.max)

        thr = {}
        masks = {}

        def build_thr(r):
            a0, a1, a2, a3 = (ak[2 * j + r] for j in range(4))
            t1 = tt_p.tile([P, OUT_PC], f32, tag="t1", name=f"t1_{r}")
            tB2 = tt_p.tile([P, OUT_PC], f32, tag="tB2", name=f"tB2_{r}")
            tC = tt_p.tile([P, OUT_PC], f32, tag="tC", name=f"tC_{r}")
            tr = thr_p.tile([P, OUT_PC], f32, tag="thr", name=f"thr_{r}")
            vtt(t1[:], tA[r][:], tB[r][:], Alu.min)
            vtt(tB2[:], a0[:], a1[:], Alu.min)
            vtt(tC[:], a2[:], a3[:], Alu.min)
            vtt(tB2[:], tB2[:], tC[:], Alu.max)
            vtt(tr[:], t1[:], tB2[:], Alu.max)
            thr[r] = tr

        def build_mask(kt):
            m = mask_p.tile([P, OUT_PC], f16, tag="mask", name=f"m{kt}")
            vtt(m[:], ak[kt][:], thr[kt % 2][:], Alu.is_ge)
            masks[kt] = m

        # range-0 tree + first mask as early as arrivals allow
        build_pair_max(0)
        build_thr(0)
        build_mask(0)
        build_pair_max(1)
        # own-half rowmaxes from the tree pair-maxes
        cm = sc_p.tile([P, 3], f32, tag="cm")
        for r in (0, 1):
            tmax = tt_p.tile([P, OUT_PC], f32, tag="tmax", name=f"tmax{r}")
            vtt(tmax[:], tA[r][:], tB[r][:], Alu.max)
            nc.vector.tensor_reduce(
                out=cm[:, r : r + 1], in_=tmax[:],
                axis=mybir.AxisListType.X, op=Alu.max,
            )
        # shadow absmax via fp16 2x max-tree, then one short reduce
        sh1 = const.tile([P, SH2], f16, tag="sh1")
        vtt(sh1[:], ash[:, 0:SH2], ash[:, SH2:SHW], Alu.max)
        sh2 = const.tile([P, SH2 // 2], f16, tag="sh2")
        vtt(sh2[:], sh1[:, 0 : SH2 // 2], sh1[:, SH2 // 2 : SH2], Alu.max)
        nc.vector.tensor_reduce(
            out=cm[:, 2:3], in_=sh2[:], axis=mybir.AxisListType.X, op=Alu.max
        )
        amc = sc_p.tile([P, 1], f32, tag="amc")
        nc.vector.reduce_max(amc[:], cm[:], axis=mybir.AxisListType.X)
        am = sc_p.tile([P, 1], f32, tag="am")
        nc.gpsimd.partition_all_reduce(
            am[:], amc[:], channels=P, reduce_op=bass_isa.ReduceOp.max
        )

        # ---- s = absmax/qmax (1 ulp); inv = 1/s (reciprocal + 1 Newton) ----
        s_t = sc_p.tile([P, 1], f32, tag="s")
        vts(s_t[:], am[:], inv_qmax, Alu.mult)
        r0 = sc_p.tile([P, 1], f32, tag="r0")
        nc.vector.reciprocal(r0[:], s_t[:])
        p1 = sc_p.tile([P, 1], f32, tag="p1")
        vtt(p1[:], s_t[:], r0[:], Alu.mult)
        e1 = sc_p.tile([P, 1], f32, tag="e1")
        vts(e1[:], p1[:], 2.0, Alu.subtract, -1.0, Alu.mult)  # 2 - s*r0
        inv_t = sc_p.tile([P, 1], f32, tag="inv")
        vtt(inv_t[:], r0[:], e1[:], Alu.mult)

        magic_t = sc_p.tile([P, 1], f32, tag="magic")
        nc.gpsimd.memset(magic_t[:], MAGIC)
        nmagic_t = sc_p.tile([P, 1], f32, tag="nmagic")
        nc.gpsimd.memset(nmagic_t[:], -MAGIC)

        # ---- quantize: k-tile 0 entirely on DVE (shortest PE gate) ----
        wqt_by_kt = {}
        q0_0 = qtmp_p.tile([P, OUT_PC], f32, tag="q0", name="q0_0")
        vts(q0_0[:], wk[0][:], inv_t[:], Alu.mult, MAGIC, Alu.add)
        q16_0 = wqt_p.tile([P, OUT_PC], f16, tag="q16", name="q16_0")
        nc.vector.scalar_tensor_tensor(
            out=q16_0[:], in0=q0_0[:], scalar=-MAGIC, in1=masks[0][:],
            op0=Alu.add, op1=Alu.mult,
        )
        wqt_by_kt[0] = q16_0

        # remaining range-0 masks, then range-1 threshold + masks (DVE)
        build_mask(2)
        build_mask(4)
        build_mask(6)
        build_thr(1)
        for kt in (1, 3, 5, 7):
            build_mask(kt)

        # k-tiles != 0: magic-round on ACT, mask-apply on Pool
        for kt in KT_ORDER[1:]:
            q0 = qtmp_p.tile([P, OUT_PC], f32, tag="q0", name=f"q0_{kt}")
            nc.scalar.activation(
                q0[:], wk[kt][:], Act.Identity, bias=magic_t[:], scale=inv_t[:]
            )
            qr = qtmp_p.tile([P, OUT_PC], f16, tag="qr", name=f"qr_{kt}")
            nc.scalar.activation(qr[:], q0[:], Act.Identity, bias=nmagic_t[:])
            q16 = wqt_p.tile([P, OUT_PC], f16, tag="q16", name=f"q16_{kt}")
            nc.gpsimd.tensor_tensor(
                out=q16[:], in0=qr[:], in1=masks[kt][:], op=Alu.mult
            )
            wqt_by_kt[kt] = q16
        wqt = [wqt_by_kt[kt] for kt in range(K_TILES)]

        # bias: unpack the fp16 columns from the shadow tensor (zeros-cheap)
        biast = const.tile([P, M_TILES], f32, tag="biast")
        nc.vector.tensor_copy(biast[:], shx[:, SHW:WXW])

        # ---- main matmul: yt[m, t] = sum_k wqt[k,m].T @ xh[k,t] ----
        # x loads on sync (fenced behind weights); evictions on DVE; y stores
        # on scalar
        for tb in range(N_TB):
            xh = [None] * K_TILES
            for ki in KT_ORDER:
                sl_p = slice(ki * P, (ki + 1) * P)
                sl_t = slice(tb * TB_TOK, (tb + 1) * TB_TOK)
                xht = x_p.tile([P, TB_TOK], f16, tag="xh", name=f"xh{tb}_{ki}")
                nc.sync.dma_start(xht[:], xth[sl_p, sl_t])
                xh[ki] = xht

            last_tb = tb == N_TB - 1

            def evict(mi, ps_tj, last_mi=False):
                ysb = y_p.tile([P, TB_TOK], f16, tag="ysb", name=f"y{tb}_{mi}")
                for tj in range(TJ):
                    dst = ysb[:, tj * MM_N : (tj + 1) * MM_N]
                    if last_mi and tj == TJ - 1:
                        # final bank: ACT, in parallel with DVE's tj0 evict
                        nc.scalar.activation(
                            dst, ps_tj[tj][:], Act.Identity,
                            bias=biast[:, mi : mi + 1], scale=s_t[:],
                        )
                    else:
                        nc.vector.tensor_scalar(
                            out=dst, in0=ps_tj[tj][:],
                            scalar1=s_t[:], scalar2=biast[:, mi : mi + 1],
                            op0=Alu.mult, op1=Alu.add,
                        )
                tcol = tb * TB_TOK
                if last_mi:
                    # split the final store so the first half leaves early
                    for tj in range(TJ):
                        nc.scalar.dma_start(
                            yt[
                                mi * P : (mi + 1) * P,
                                tcol + tj * MM_N : tcol + (tj + 1) * MM_N,
                            ],
                            ysb[:, tj * MM_N : (tj + 1) * MM_N],
                        )
                else:
                    nc.scalar.dma_start(
                        yt[mi * P : (mi + 1) * P, tcol : tcol + TB_TOK], ysb[:]
                    )

            if tb == 0:
                # k-outer sweep over all 4 m-tiles (8 PSUM banks): PE starts
                # on the first quantized k-tile, consuming at the prep pace
                ps = {
                    (mi, tj): psum_mm.tile(
                        [P, MM_N], f32, tag="ps", name=f"ps0_{mi}_{tj}"
                    )
                    for mi in range(M_TILES)
                    for tj in range(TJ)
                }
                for kpos, ki in enumerate(KT_ORDER):
                    for mi in range(M_TILES):
                        lhsT = wqt[ki][:, mi * P : (mi + 1) * P]
                        for tj in range(TJ):
                            nc.tensor.matmul(
                                ps[mi, tj][:],
                                lhsT,
                                xh[ki][:, tj * MM_N : (tj + 1) * MM_N],
                                start=(kpos == 0),
                                stop=(kpos == K_TILES - 1),
                            )
                for mi in range(M_TILES):
                    evict(mi, [ps[mi, tj] for tj in range(TJ)])
            else:
                for mi in range(M_TILES):
                    ps = [
                        psum_mm.tile(
                            [P, MM_N], f32, tag="ps", name=f"ps{tb}_{mi}_{tj}"
                        )
                        for tj in range(TJ)
                    ]
                    for kpos, ki in enumerate(KT_ORDER):
                        lhsT = wqt[ki][:, mi * P : (mi + 1) * P]
                        for tj in range(TJ):
                            nc.tensor.matmul(
                                ps[tj][:],
                                lhsT,
                                xh[ki][:, tj * MM_N : (tj + 1) * MM_N],
                                start=(kpos == 0),
                                stop=(kpos == K_TILES - 1),
                            )
                    evict(mi, ps, last_mi=last_tb and mi == M_TILES - 1)

    nc.compile()
    return nc


def _get(qmax: float):
    key = qmax
    if key not in _CACHE:
        _CACHE[key] = _build(qmax)
    return _CACHE[key]


def host_prep(x, weight):
    """Host-side input re-encoding: transpose, phase-major permute the in_f
    axis, fp16 encodes, and pack the shadow/bias layouts. Pure layout."""
    xt = np.ascontiguousarray(x.T)[_PERM]  # [IN_F perm, TOKENS]
    xth = xt.astype(np.float16)
    wp = np.ascontiguousarray(weight.T[_PERM])  # [IN_F perm, OUT_F] fp32
    wp16 = wp.astype(np.float16)
    return xth, wp, wp16


LAST_EXEC_NS = None


def kernel(x, weight, bias, precision, _trace_dir=None):
    global LAST_EXEC_NS
    from concourse.bass_utils import run_bass_kernel_spmd

    x = np.asarray(x, dtype=np.float32)
    weight = np.asarray(weight, dtype=np.float32)
    bias = np.asarray(bias, dtype=np.float32)
    prec = int(np.asarray(precision))
    qmax = float(2 ** (prec - 1) - 1)

    nc = _get(qmax)

    xth, wp, wp16 = host_prep(x, weight)
    in_maps = []
    for c in range(N_CORES):
        tg, fg = c // F_GROUPS, c % F_GROUPS
        o0, o1 = fg * OUT_PC, (fg + 1) * OUT_PC
        x0, x1 = (1 - fg) * OUT_PC, (2 - fg) * OUT_PC
        shadow = wp16[:, x0:x1]  # [1024, 512] fp16, other half
        wpx_packed = np.empty((P, WXW), dtype=np.float16)
        wpx_packed[:, :SHW] = (
            shadow.reshape(K_TILES, P, OUT_PC).transpose(1, 0, 2).reshape(P, SHW)
        )
        # fp16-packed bias columns (bias is tiny; fp16 rounding ~2^-11)
        wpx_packed[:, SHW:] = (
            bias[o0:o1].reshape(M_TILES, P).T.astype(np.float16)
        )
        in_maps.append(
            {
                "xth": np.ascontiguousarray(
                    xth[:, tg * TOK_PC : (tg + 1) * TOK_PC]
                ),
                "wpo": np.ascontiguousarray(wp[:, o0:o1]),
                "wpx": wpx_packed,
            }
        )
    kw = {}
    if _trace_dir is not None:
        kw = {"trace": True, "tmpdir": _trace_dir}
    res = run_bass_kernel_spmd(nc, in_maps, list(range(N_CORES)), **kw)
    LAST_EXEC_NS = res.exec_time_ns
    y = np.empty((TOKENS, OUT_F), dtype=np.float32)
    for c in range(N_CORES):
        tg, fg = c // F_GROUPS, c % F_GROUPS
        y[tg * TOK_PC : (tg + 1) * TOK_PC, fg * OUT_PC : (fg + 1) * OUT_PC] = (
            res.results[c]["yt"].T.astype(np.float32)
        )
    return y


# revision 26
# speedup vs baseline: 1.0411x; 1.0411x over previous
"""BlackwellLinear Trainium2 kernel: 2:4 sparsity + int8 fake-quant + x @ w.T + bias.

Full inputs in, full output out. Hybrid sharding across 8 NeuronCores:
4 token groups x 2 out_feature groups. Each core computes
y[tg-block, fg-block] = x[tg] @ w[fg].T * scale + bias[fg], and runs the
module's weight prep (2:4 sparsify + int8 fake-quant) for its own out_feature
half -- halving the elementwise prep work per core vs pure data-parallel
(that prep chain gates the single-pass matmul pipeline start). No collectives:
the global absmax over the sparsified weight equals absmax of |w| (the global
max always survives top-2-of-4 selection), and each core computes it from its
own fp32 half plus a compact fp16 shadow of the other half (scale perturbation
~2^-11, far inside the error budget).

Host does layout/encoding only: transposes, fp16 encodes of x / shadow / bias,
and a phase-major permutation of the in_features axis
(p <-> 4*(p%256) + p//256) applied to both x.T and w.T. The permutation makes
each group-of-4 (the 2:4 unit) span four k-tiles at the SAME partition/column
coordinates, so sparsify+quantize is contiguous full-width elementwise ops and
the quantized weight lands directly in [in_f, out_f] (lhsT) layout. All module
math (threshold, mask, quantize, matmul, bias) runs on device.

Engine layout of the prep, tuned from traces:
 - All weight DMAs issue before anything else; x-strip loads are fenced behind
   the last weight tile by a tiny dependent copy so they cannot crowd the wire
   during the prep-critical weight load. Bias rides inside the shadow tensor
   (fp16) so no separate DMA can get hoisted ahead of weights.
 - ACT: |w| for range-0 tiles, |shadow|, magic-round pairs for k-tiles != 0.
 - Pool: w*w for range-1 tiles (squares are order-isomorphic to |w| and
   fp32-tie-safe), mask-apply multiplies, the partition allreduce.
 - DVE: threshold trees (range 1 in the squared domain), masks (exact fp32
   compares -> reference-identical 2:4 tie behavior), absmax reduces (shadow
   via fp16 2x max-tree), PSUM evictions, and the WHOLE quantize chain for
   k-tile 0 (no cross-engine hops on the PE-gating tile).
 - PE: one fp16 pass -- 512 MMs of N=512, the dense roofline.

Numerics (harness gate rel_err < 2e-2; this kernel lands ~1e-3):
  s   = absmax * (1/qmax)                  (1 ulp from fl(absmax/qmax))
  inv ~= 1/s                               (HW reciprocal + 1 Newton step)
  k   = rne(w * inv)                       (magic-constant RNE round)
  q   = k * mask                           (2:4 mask; integers are fp16-exact)
  y   = s * (x16 @ q.T) + bias             (scale folded into PSUM eviction)
"""

import numpy as np

N_CORES = 8
P = 128
IN_F = 1024
OUT_F = 1024
TOKENS = 32768
T_GROUPS = 4
F_GROUPS = 2
TOK_PC = TOKENS // T_GROUPS  # 8192 tokens per core
OUT_PC = OUT_F // F_GROUPS  # 512 out_features per core
K_TILES = IN_F // P  # 8
M_TILES = OUT_PC // P  # 4
TB_TOK = 1024  # token block per x strip
N_TB = TOK_PC // TB_TOK  # 8
MM_N = 512  # matmul moving free dim (one PSUM bank of fp32)
TJ = TB_TOK // MM_N  # 2
SHW = K_TILES * OUT_PC  # 4096 shadow columns
WXW = SHW + M_TILES  # + fp16-packed bias columns

MAGIC = 12582912.0  # 1.5 * 2**23: (v + MAGIC) - MAGIC == RNE round, |v| <= 2**22

# k-tile order: range-0 tiles (phases of groups 0..127) first so the range-0
# threshold -> mask -> quant chain completes with half the weight DMA landed;
# PE accumulates k-tiles in this same order (sum order is commutative).
KT_ORDER = (0, 2, 4, 6, 1, 3, 5, 7)

# phase-major permutation of the in_features axis: position p holds original
# feature 4*(p%256) + p//256, so k-tile kt covers phase kt//2 of group range
# (kt%2)*128..+128 and the four phases of a group share partition/column coords
_PERM = (4 * (np.arange(IN_F) % 256) + np.arange(IN_F) // 256).astype(np.int64)

_CACHE = {}


def _build(qmax: float):
    from contextlib import ExitStack

    import concourse.tile as tile
    import concourse.mybir as mybir
    from concourse import bacc, bass_isa

    f32 = mybir.dt.float32
    f16 = mybir.dt.float16
    Alu = mybir.AluOpType
    Act = mybir.ActivationFunctionType

    inv_qmax = float(np.float32(1.0) / np.float32(qmax))

    nc = bacc.Bacc("TRN2", target_bir_lowering=False, debug=False)
    xth = nc.dram_tensor("xth", [IN_F, TOK_PC], f16, kind="ExternalInput").ap()
    # own out_f half of w.T (permuted in_f rows), fp32: exact 2:4 tie behavior
    wpo = nc.dram_tensor("wpo", [IN_F, OUT_PC], f32, kind="ExternalInput").ap()
    # other half, fp16 shadow packed [128, 4096] + fp16 bias [128, M_TILES]
    wpx = nc.dram_tensor("wpx", [P, WXW], f16, kind="ExternalInput").ap()
    yt = nc.dram_tensor("yt", [OUT_PC, TOK_PC], f16, kind="ExternalOutput").ap()

    with tile.TileContext(nc) as tc, ExitStack() as ctx:
        const = ctx.enter_context(tc.tile_pool(name="const", bufs=1))
        wnat_p = ctx.enter_context(tc.tile_pool(name="wnat", bufs=8))
        abs_p = ctx.enter_context(tc.tile_pool(name="absp", bufs=8))
        thr_p = ctx.enter_context(tc.tile_pool(name="thr", bufs=2))
        tt_p = ctx.enter_context(tc.tile_pool(name="ttmp", bufs=2))
        mask_p = ctx.enter_context(tc.tile_pool(name="mask", bufs=8))
        qtmp_p = ctx.enter_context(tc.tile_pool(name="qtmp", bufs=2))
        wqt_p = ctx.enter_context(tc.tile_pool(name="wqt", bufs=8))
        sc_p = ctx.enter_context(tc.tile_pool(name="sc", bufs=1))
        x_p = ctx.enter_context(tc.tile_pool(name="x", bufs=16))
        y_p = ctx.enter_context(tc.tile_pool(name="y", bufs=4))
        psum_mm = ctx.enter_context(tc.tile_pool(name="psmm", bufs=8, space="PSUM"))

        # ---- PE pre-warm: one long accumulation group of zeros holds HAM at
        # full clock through the prep phase. Trace-calibrated: warmup begins
        # ~7.8us, prep-ready is ~35.3us; 110 MMs = ~3.4us cold + 102*216ns
        # ends ~33.4us, <3.4us before the first real matmul (no re-throttle,
        # no overshoot). If prep runs late the PE re-throttles and we merely
        # fall back to v8's cold-start behavior. ----
        N_WARM = 110
        zwarm = const.tile([P, MM_N], f16, tag="zwarm")
        nc.gpsimd.memset(zwarm[:], 0.0)
        ps_w = psum_mm.tile([P, MM_N], f32, tag="ps", name="ps_warm")
        for i in range(N_WARM):
            nc.tensor.matmul(
                ps_w[:], zwarm[:, 0:P], zwarm[:], start=(i == 0),
                stop=(i == N_WARM - 1),
            )

        # ---- ALL weight DMAs first, shadow halves split across queues ----
        shx = const.tile([P, WXW], f16, tag="shx")
        SH2 = SHW // 2
        wk = [None] * K_TILES

        def wdma(eng, kt):
            wt = wnat_p.tile([P, OUT_PC], f32, tag="wnat", name=f"wnat{kt}")
            eng.dma_start(wt[:], wpo[kt * P : (kt + 1) * P, :])
            wk[kt] = wt

        wdma(nc.sync, 0)
        wdma(nc.scalar, 1)
        wdma(nc.sync, 2)
        wdma(nc.scalar, 3)
        wdma(nc.sync, 4)
        wdma(nc.scalar, 5)
        wdma(nc.sync, 6)
        wdma(nc.scalar, 7)
        nc.sync.dma_start(shx[:, 0:SH2], wpx[:, 0:SH2])
        nc.sync.dma_start(shx[:, SH2:WXW], wpx[:, SH2:WXW])

        # fence: x loads (emitted later on sync) stay behind the last weight
        # tile on the wire -- this copy cannot issue until wnat7 has landed
        gate = const.tile([1, 8], f32, tag="gate")
        nc.sync.dma_start(gate[:, :], wk[7][0:1, 0:8])

        # ---- ACT: |w| per k-tile (arrival-pipelined), then |shadow| ----
        ak = [None] * K_TILES
        for kt in KT_ORDER:
            a = abs_p.tile([P, OUT_PC], f32, tag="abs", name=f"abs{kt}")
            nc.scalar.activation(a[:], wk[kt][:], Act.Abs)
            ak[kt] = a
        ash = const.tile([P, SHW], f16, tag="ash")
        nc.scalar.activation(ash[:, 0:SH2], shx[:, 0:SH2], Act.Abs)
        nc.scalar.activation(ash[:, SH2:SHW], shx[:, SH2:SHW], Act.Abs)

        def vts(out, in0, s1, op0, s2=None, op1=None):
            kw = {"op1": op1} if op1 is not None else {}
            nc.vector.tensor_scalar(
                out=out, in0=in0, scalar1=s1, scalar2=s2, op0=op0, **kw
            )

        def vtt(out, in0, in1, op):
            nc.vector.tensor_tensor(out=out, in0=in0, in1=in1, op=op)

        # ---- DVE chain (ordered for earliest inv -> q16[kt0]) ----
        tA, tB = {}, {}

        def build_pair_max(r):
            a0, a1, a2, a3 = (ak[2 * j + r] for j in range(4))
            tA[r] = tt_p.tile([P, OUT_PC], f32, tag="tA", name=f"tA{r}")
            tB[r] = tt_p.tile([P, OUT_PC], f32, tag="tB", name=f"tB{r}")
            vtt(tA[r][:], a0[:], a1[:], Alu.max)
            vtt(tB[r][:], a2[:], a3[:], Alu.max)

        thr = {}
        masks = {}

        def build_thr(r):
            a0, a1, a2, a3 = (ak[2 * j + r] for j in range(4))
            t1 = tt_p.tile([P, OUT_PC], f32, tag="t1", name=f"t1_{r}")
            tB2 = tt_p.tile([P, OUT_PC], f32, tag="tB2", name=f"tB2_{r}")
            tC = tt_p.tile([P, OUT_PC], f32, tag="tC", name=f"tC_{r}")
            tr = thr_p.tile([P, OUT_PC], f32, tag="thr", name=f"thr_{r}")
            vtt(t1[:], tA[r][:], tB[r][:], Alu.min)
            vtt(tB2[:], a0[:], a1[:], Alu.min)
            vtt(tC[:], a2[:], a3[:], Alu.min)
            vtt(tB2[:], tB2[:], tC[:], Alu.max)
            vtt(tr[:], t1[:], tB2[:], Alu.max)
            thr[r] = tr

        def build_mask(kt):
            m = mask_p.tile([P, OUT_PC], f16, tag="mask", name=f"m{kt}")
            vtt(m[:], ak[kt][:], thr[kt % 2][:], Alu.is_ge)
            masks[kt] = m

        # range-0 tree + first mask as early as arrivals allow
        build_pair_max(0)
        build_thr(0)
        build_mask(0)
        build_pair_max(1)
        # own-half rowmaxes from the tree pair-maxes
        cm = sc_p.tile([P, 3], f32, tag="cm")
        for r in (0, 1):
            tmax = tt_p.tile([P, OUT_PC], f32, tag="tmax", name=f"tmax{r}")
            vtt(tmax[:], tA[r][:], tB[r][:], Alu.max)
            nc.vector.tensor_reduce(
                out=cm[:, r : r + 1], in_=tmax[:],
                axis=mybir.AxisListType.X, op=Alu.max,
            )
        # shadow absmax via fp16 2x max-tree, then one short reduce
        sh1 = const.tile([P, SH2], f16, tag="sh1")
        vtt(sh1[:], ash[:, 0:SH2], ash[:, SH2:SHW], Alu.max)
        sh2 = const.tile([P, SH2 // 2], f16, tag="sh2")
        vtt(sh2[:], sh1[:, 0 : SH2 // 2], sh1[:, SH2 // 2 : SH2], Alu.max)
        nc.vector.tensor_reduce(
            out=cm[:, 2:3], in_=sh2[:], axis=mybir.AxisListType.X, op=Alu.max
        )
        amc = sc_p.tile([P, 1], f32, tag="amc")
        nc.vector.reduce_max(amc[:], cm[:], axis=mybir.AxisListType.X)
        am = sc_p.tile([P, 1], f32, tag="am")
        nc.gpsimd.partition_all_reduce(
            am[:], amc[:], channels=P, reduce_op=bass_isa.ReduceOp.max
        )

        # ---- s = absmax/qmax (1 ulp); inv = 1/s (reciprocal + 1 Newton) ----
        s_t = sc_p.tile([P, 1], f32, tag="s")
        vts(s_t[:], am[:], inv_qmax, Alu.mult)
        r0 = sc_p.tile([P, 1], f32, tag="r0")
        nc.vector.reciprocal(r0[:], s_t[:])
        p1 = sc_p.tile([P, 1], f32, tag="p1")
        vtt(p1[:], s_t[:], r0[:], Alu.mult)
        e1 = sc_p.tile([P, 1], f32, tag="e1")
        vts(e1[:], p1[:], 2.0, Alu.subtract, -1.0, Alu.mult)  # 2 - s*r0
        inv_t = sc_p.tile([P, 1], f32, tag="inv")
        vtt(inv_t[:], r0[:], e1[:], Alu.mult)

        magic_t = sc_p.tile([P, 1], f32, tag="magic")
        nc.gpsimd.memset(magic_t[:], MAGIC)
        nmagic_t = sc_p.tile([P, 1], f32, tag="nmagic")
        nc.gpsimd.memset(nmagic_t[:], -MAGIC)

        # ---- quantize: k-tile 0 entirely on DVE (shortest PE gate) ----
        wqt_by_kt = {}
        q0_0 = qtmp_p.tile([P, OUT_PC], f32, tag="q0", name="q0_0")
        vts(q0_0[:], wk[0][:], inv_t[:], Alu.mult, MAGIC, Alu.add)
        q16_0 = wqt_p.tile([P, OUT_PC], f16, tag="q16", name="q16_0")
        nc.vector.scalar_tensor_tensor(
            out=q16_0[:], in0=q0_0[:], scalar=-MAGIC, in1=masks[0][:],
            op0=Alu.add, op1=Alu.mult,
        )
        wqt_by_kt[0] = q16_0

        # remaining range-0 masks, then range-1 threshold + masks (DVE)
        build_mask(2)
        build_mask(4)
        build_mask(6)
        build_thr(1)
        for kt in (1, 3, 5, 7):
            build_mask(kt)

        # k-tiles != 0: magic-round on ACT, mask-apply on Pool
        for kt in KT_ORDER[1:]:
            q0 = qtmp_p.tile([P, OUT_PC], f32, tag="q0", name=f"q0_{kt}")
            nc.scalar.activation(
                q0[:], wk[kt][:], Act.Identity, bias=magic_t[:], scale=inv_t[:]
            )
            qr = qtmp_p.tile([P, OUT_PC], f16, tag="qr", name=f"qr_{kt}")
            nc.scalar.activation(qr[:], q0[:], Act.Identity, bias=nmagic_t[:])
            q16 = wqt_p.tile([P, OUT_PC], f16, tag="q16", name=f"q16_{kt}")
            nc.gpsimd.tensor_tensor(
                out=q16[:], in0=qr[:], in1=masks[kt][:], op=Alu.mult
            )
            wqt_by_kt[kt] = q16
        wqt = [wqt_by_kt[kt] for kt in range(K_TILES)]

        # bias: unpack the fp16 columns from the shadow tensor (zeros-cheap)
        biast = const.tile([P, M_TILES], f32, tag="biast")
        nc.vector.tensor_copy(biast[:], shx[:, SHW:WXW])

        # ---- main matmul: yt[m, t] = sum_k wqt[k,m].T @ xh[k,t] ----
        # x loads on sync (fenced behind weights); evictions on DVE; y stores
        # on scalar
        for tb in range(N_TB):
            xh = [None] * K_TILES
            for ki in KT_ORDER:
                sl_p = slice(ki * P, (ki + 1) * P)
                sl_t = slice(tb * TB_TOK, (tb + 1) * TB_TOK)
                xht = x_p.tile([P, TB_TOK], f16, tag="xh", name=f"xh{tb}_{ki}")
                nc.sync.dma_start(xht[:], xth[sl_p, sl_t])
                xh[ki] = xht

            last_tb = tb == N_TB - 1

            def evict(mi, ps_tj, last_mi=False):
                ysb = y_p.tile([P, TB_TOK], f16, tag="ysb", name=f"y{tb}_{mi}")
                for tj in range(TJ):
                    dst = ysb[:, tj * MM_N : (tj + 1) * MM_N]
                    if last_mi and tj == TJ - 1:
                        # final bank: ACT, in parallel with DVE's tj0 evict
                        nc.scalar.activation(
                            dst, ps_tj[tj][:], Act.Identity,
                            bias=biast[:, mi : mi + 1], scale=s_t[:],
                        )
                    else:
                        nc.vector.tensor_scalar(
                            out=dst, in0=ps_tj[tj][:],
                            scalar1=s_t[:], scalar2=biast[:, mi : mi + 1],
                            op0=Alu.mult, op1=Alu.add,
                        )
                tcol = tb * TB_TOK
                if last_mi:
                    # split the final store so the first half leaves early
                    for tj in range(TJ):
                        nc.scalar.dma_start(
                            yt[
                                mi * P : (mi + 1) * P,
                                tcol + tj * MM_N : tcol + (tj + 1) * MM_N,
                            ],
                            ysb[:, tj * MM_N : (tj + 1) * MM_N],
                        )
                else:
                    nc.scalar.dma_start(
                        yt[mi * P : (mi + 1) * P, tcol : tcol + TB_TOK], ysb[:]
                    )

            if tb == 0:
                # k-outer sweep over all 4 m-tiles (8 PSUM banks): PE starts
                # on the first quantized k-tile, consuming at the prep pace
                ps = {
                    (mi, tj): psum_mm.tile(
                        [P, MM_N], f32, tag="ps", name=f"ps0_{mi}_{tj}"
                    )
                    for mi in range(M_TILES)
                    for tj in range(TJ)
                }
                for kpos, ki in enumerate(KT_ORDER):
                    for mi in range(M_TILES):
                        lhsT = wqt[ki][:, mi * P : (mi + 1) * P]
                        for tj in range(TJ):
                            nc.tensor.matmul(
                                ps[mi, tj][:],
                                lhsT,
                                xh[ki][:, tj * MM_N : (tj + 1) * MM_N],
                                start=(kpos == 0),
                                stop=(kpos == K_TILES - 1),
                            )
                for mi in range(M_TILES):
                    evict(mi, [ps[mi, tj] for tj in range(TJ)])
            else:
                for mi in range(M_TILES):
                    ps = [
                        psum_mm.tile(
                            [P, MM_N], f32, tag="ps", name=f"ps{tb}_{mi}_{tj}"
                        )
                        for tj in range(TJ)
                    ]
                    for kpos, ki in enumerate(KT_ORDER):
                        lhsT = wqt[ki][:, mi * P : (mi + 1) * P]
                        for tj in range(TJ):
                            nc.tensor.matmul(
                                ps[tj][:],
                                lhsT,
                                xh[ki][:, tj * MM_N : (tj + 1) * MM_N],
                                start=(kpos == 0),
                                stop=(kpos == K_TILES - 1),
                            )
                    evict(mi, ps, last_mi=last_tb and mi == M_TILES - 1)

    nc.compile()
    return nc


def _get(qmax: float):
    key = qmax
    if key not in _CACHE:
        _CACHE[key] = _build(qmax)
    return _CACHE[key]


def host_prep(x, weight):
    """Host-side input re-encoding: transpose, phase-major permute the in_f
    axis, fp16 encodes, and pack the shadow/bias layouts. Pure layout."""
    xt = np.ascontiguousarray(x.T)[_PERM]  # [IN_F perm, TOKENS]
    xth = xt.astype(np.float16)
    wp = np.ascontiguousarray(weight.T[_PERM])  # [IN_F perm, OUT_F] fp32
    wp16 = wp.astype(np.float16)
    return xth, wp, wp16


LAST_EXEC_NS = None


def kernel(x, weight, bias, precision, _trace_dir=None):
    global LAST_EXEC_NS
    from concourse.bass_utils import run_bass_kernel_spmd

    x = np.asarray(x, dtype=np.float32)
    weight = np.asarray(weight, dtype=np.float32)
    bias = np.asarray(bias, dtype=np.float32)
    prec = int(np.asarray(precision))
    qmax = float(2 ** (prec - 1) - 1)

    nc = _get(qmax)

    xth, wp, wp16 = host_prep(x, weight)
    in_maps = []
    for c in range(N_CORES):
        tg, fg = c // F_GROUPS, c % F_GROUPS
        o0, o1 = fg * OUT_PC, (fg + 1) * OUT_PC
        x0, x1 = (1 - fg) * OUT_PC, (2 - fg) * OUT_PC
        shadow = wp16[:, x0:x1]  # [1024, 512] fp16, other half
        wpx_packed = np.empty((P, WXW), dtype=np.float16)
        wpx_packed[:, :SHW] = (
            shadow.reshape(K_TILES, P, OUT_PC).transpose(1, 0, 2).reshape(P, SHW)
        )
        # fp16-packed bias columns (bias is tiny; fp16 rounding ~2^-11)
        wpx_packed[:, SHW:] = (
            bias[o0:o1].reshape(M_TILES, P).T.astype(np.float16)
        )
        in_maps.append(
            {
                "xth": np.ascontiguousarray(
                    xth[:, tg * TOK_PC : (tg + 1) * TOK_PC]
                ),
                "wpo": np.ascontiguousarray(wp[:, o0:o1]),
                "wpx": wpx_packed,
            }
        )
    kw = {}
    if _trace_dir is not None:
        kw = {"trace": True, "tmpdir": _trace_dir}
    res = run_bass_kernel_spmd(nc, in_maps, list(range(N_CORES)), **kw)
    LAST_EXEC_NS = res.exec_time_ns
    y = np.empty((TOKENS, OUT_F), dtype=np.float32)
    for c in range(N_CORES):
        tg, fg = c // F_GROUPS, c % F_GROUPS
        y[tg * TOK_PC : (tg + 1) * TOK_PC, fg * OUT_PC : (fg + 1) * OUT_PC] = (
            res.results[c]["yt"].T.astype(np.float32)
        )
    return y
